# revision 24
# baseline (speedup 1.0000x reference)
"""Trainium2 Bass kernel for a custom Jacobi-basis layer.

Math:
    t = tanh(x)                                  x: [B, I] f32
    J[b,i,k] = P_k^(1,1)(t[b,i])                 Jacobi polys, k = 0..8
    out[b,o] = sum_{i,k} J[b,i,k] * coeff[o,i,k] * weights[o,i]

Strategy (8 NeuronCores, data-parallel over batch):
  * Fold weights into coeff on host: Cw[o,i,k] = coeff[o,i,k]*weights[o,i].
  * alpha=beta=1 makes the three-term recurrence two-term coefficient-free
    after rescaling: G_1 = t, G_k = t*G_{k-1} - B'_k*G_{k-2} with G_k = c_k*J_k.
    The 1/c_k scale is folded into the (host-prepared) matmul operand.
  * J_0 == 1, so the k=0 term is a per-output bias, applied with a K=1 matmul.
  * Per core: tanh/square on ScalarE, fp32 recurrence on VectorE (fused
    scalar_tensor_tensor ops), then 128 float32r matmuls [128x128]@[128x512]
    accumulating in PSUM over the 4096-long (i,k) contraction.
    float32r runs at bf16 speed for N>=256 but has ~1.5e-4 matmul error
    (vs 2.3e-3 for bf16).
"""

import numpy as np

import concourse.mybir as mybir
import concourse.tile as tile
from concourse import bacc
from concourse.bass_utils import run_bass_kernel_spmd

ORDER = 8
B, I, O = 4096, 512, 512
NCORES = 8
BC = B // NCORES          # batch rows per core = 512
P = 128                   # partitions
NIC = I // P              # i-chunks = 4
BT = BC // P              # b-tiles per core = 4
FREE = NIC * BC           # free dim of basis planes = 2048


def _consts():
    """Recurrence constants (alpha=beta=1, so the k2 term is 0)."""
    a = b = 1.0
    A, Bk = {}, {}
    for i in range(2, ORDER + 1):
        A[i] = (2 * i + a + b) * (2 * i + a + b - 1) / (2 * i * (i + a + b))
        Bk[i] = (i + a - 1) * (i + b - 1) * (2 * i + a + b) / (
            i * (i + a + b) * (2 * i + a + b - 2)
        )
    c = {0: 1.0, 1: 0.5}
    for i in range(2, ORDER + 1):
        c[i] = c[i - 1] / A[i]
    Bp = {i: Bk[i] * c[i] / c[i - 2] for i in range(2, ORDER + 1)}
    return c, Bp


def _build_module():
    nc = bacc.Bacc("TRN2", num_devices=NCORES)
    f32 = mybir.dt.float32
    f32r = mybir.dt.float32r

    xt_d = nc.dram_tensor("xt", [P, FREE], f32, kind="ExternalInput")
    # r layout: [p, (k-1)*FREE + ic*O + o] = Cw[o, ic*128+p, k] / c_k
    r_d = nc.dram_tensor("r", [P, ORDER * FREE], f32r, kind="ExternalInput")
    # consts = [ones(128) | bias(512)]
    consts_d = nc.dram_tensor("consts", [1, P + O], f32r, kind="ExternalInput")
    # out layout: [p, bt*O + o] = output[core*BC + bt*128 + p, o]
    out_d = nc.dram_tensor("out", [P, BT * O], f32, kind="ExternalOutput")

    _, Bp = _consts()
    mult = mybir.AluOpType.mult
    add = mybir.AluOpType.add

    from concourse.tile_rust import add_dep_helper

    with tile.TileContext(nc) as tc:
        with (
            tc.tile_pool(name="io", bufs=1) as io,
            tc.tile_pool(name="g", bufs=1) as gp,
            tc.tile_pool(name="u", bufs=2) as up,
            tc.tile_pool(name="psum", bufs=1, space="PSUM") as pp,
        ):
            # DMAs, chained so early-needed transfers get full bandwidth:
            # xt -> consts -> r0 -> r1 -> ... -> r7
            x_t = io.tile([P, FREE], f32, tag="x")
            d_prev = nc.sync.dma_start(x_t[:], xt_d[:])
            const_t = io.tile([1, P + O], f32r, tag="consts")
            d = nc.sync.dma_start(const_t[:], consts_d[:])
            add_dep_helper(d.ins, d_prev.ins, reason="dma priority chain")
            d_prev = d
            ones_t = const_t[:, 0:P]
            bias_t = const_t[:, P : P + O]
            r_t = []
            for k in range(ORDER):
                rt = io.tile([P, FREE], f32r, tag=f"r{k}", name=f"r{k}")
                d = nc.sync.dma_start(rt[:], r_d[:, k * FREE : (k + 1) * FREE])
                add_dep_helper(d.ins, d_prev.ins, reason="dma priority chain")
                d_prev = d
                r_t.append(rt)

            # Basis planes G_1..G_8: recurrence in fp32 (DVE), then one
            # rounding cast per plane to f32r on the (otherwise idle) ScalarE.
            g = [None] * (ORDER + 1)   # fp32 recurrence planes
            gr = [None] * (ORDER + 1)  # f32r matmul operand planes
            t = gp.tile([P, FREE], f32, tag="t")
            nc.scalar.activation(t[:], x_t[:], mybir.ActivationFunctionType.Tanh)
            g[1] = t
            sq = up.tile([P, FREE], f32, tag="sq")
            nc.scalar.square(sq[:], t[:])
            g2 = gp.tile([P, FREE], f32, tag="g", name="g2", bufs=3)
            nc.vector.tensor_scalar_add(g2[:], sq[:], -Bp[2])
            g[2] = g2
            for k in range(3, ORDER + 1):
                u = up.tile([P, FREE], f32, tag="u", name=f"u{k}")
                nc.vector.tensor_tensor(u[:], t[:], g[k - 1][:], mult)
                gk = gp.tile([P, FREE], f32, tag="g", name=f"g{k}", bufs=3)
                nc.vector.scalar_tensor_tensor(
                    gk[:], g[k - 2][:], -Bp[k], u[:], mult, add
                )
                g[k] = gk
            for k in range(1, ORDER + 1):
                grk = gp.tile([P, FREE], f32r, tag="gr", name=f"gr{k}", bufs=4)
                nc.scalar.copy(grk[:], g[k][:])
                gr[k] = grk

            # Matmuls: psum[bt] = ones^T @ bias + sum_{k,ic} G_k_slice^T @ R_k_slice
            psums = [
                pp.tile([P, O], f32, tag=f"ps{bt}", name=f"ps{bt}")
                for bt in range(BT)
            ]
            for bt in range(BT):
                nc.tensor.matmul(
                    psums[bt][:], ones_t, bias_t, start=True, stop=False
                )
            out_t = io.tile([P, BT * O], f32, tag="out")
            for k in range(1, ORDER + 1):
                if k < ORDER:
                    for ic in range(NIC):
                        for bt in range(BT):
                            col = ic * BC + bt * P
                            nc.tensor.matmul(
                                psums[bt][:],
                                gr[k][:, col : col + P],
                                r_t[k - 1][:, ic * O : (ic + 1) * O],
                                start=False,
                                stop=False,
                            )
                else:
                    # last contraction block: finish b-tiles one at a time so
                    # the psum evictions/stores overlap the remaining matmuls
                    for bt in range(BT):
                        for ic in range(NIC):
                            col = ic * BC + bt * P
                            nc.tensor.matmul(
                                psums[bt][:],
                                gr[k][:, col : col + P],
                                r_t[k - 1][:, ic * O : (ic + 1) * O],
                                start=False,
                                stop=ic == NIC - 1,
                            )
                        dst = out_t[:, bt * O : (bt + 1) * O]
                        if bt % 2 == 0:
                            nc.scalar.copy(dst, psums[bt][:])
                        else:
                            nc.vector.tensor_copy(dst, psums[bt][:])
                        nc.gpsimd.dma_start(
                            out_d[:, bt * O : (bt + 1) * O],
                            out_t[:, bt * O : (bt + 1) * O],
                        )
    nc.compile()
    return nc


def _prep_operands(weights, coeff):
    """Host-side, input-independent preprocessing of the layer constants."""
    c, _ = _consts()
    Cw = coeff.astype(np.float64) * weights.astype(np.float64)[:, :, None]
    bias = Cw[:, :, 0].sum(axis=1)                      # [O]
    r = np.empty((ORDER, P, FREE), dtype=np.float32)
    for k in range(1, ORDER + 1):
        tmp = (Cw[:, :, k] / c[k]).T.astype(np.float32)  # [I, O]
        r[k - 1] = tmp.reshape(NIC, P, O).transpose(1, 0, 2).reshape(P, FREE)
    r = np.ascontiguousarray(r.transpose(1, 0, 2).reshape(P, ORDER * FREE))
    consts = np.concatenate(
        [np.ones(P, dtype=np.float32), bias.astype(np.float32)]
    ).reshape(1, P + O)
    return r, consts


def _prep_x(x):
    """Per-core [128, FREE] views of x^T: xt[p, ic*BC + b] = x[core*BC+b, ic*128+p]."""
    shards = []
    for core in range(NCORES):
        xc = np.ascontiguousarray(x[core * BC : (core + 1) * BC, :].T)  # [I, BC]
        shards.append(
            np.ascontiguousarray(
                xc.reshape(NIC, P, BC).transpose(1, 0, 2).reshape(P, FREE)
            )
        )
    return shards


def _install_ntff_hook():
    """Register the NTFF profile hook that the image's boot skips (no
    antenv.axon_hooks module). Same ctypes ABI as trn_boot's
    _ntff_profile_via_ctypes. Only used for traced (profiling) runs."""
    import sys
    import types
    import ctypes
    import contextlib

    if "antenv.axon_hooks" in sys.modules:
        return
    mod = types.ModuleType("antenv.axon_hooks")
    state = {"hook": None}
    mod.set_axon_ntff_profile_hook = lambda h: state.__setitem__("hook", h)
    mod.get_axon_ntff_profile_hook = lambda: state["hook"]
    sys.modules["antenv.axon_hooks"] = mod
    import antenv

    antenv.axon_hooks = mod

    so_path = "/opt/axon/libaxon_pjrt.so"
    lib = ctypes.CDLL(so_path)
    if not hasattr(lib, "axon_start_nrt_profile"):
        return
    lib.axon_start_nrt_profile.argtypes = [
        ctypes.POINTER(ctypes.c_int64),
        ctypes.c_size_t,
    ]
    lib.axon_start_nrt_profile.restype = ctypes.c_int64
    lib.axon_stop_nrt_profile.argtypes = [ctypes.c_char_p]
    lib.axon_stop_nrt_profile.restype = ctypes.c_int64

    @contextlib.contextmanager
    def _hook(output_dir, device_ids):
        import jax

        jax.devices()
        if device_ids:
            ids = (ctypes.c_int64 * len(device_ids))(*device_ids)
            rc = lib.axon_start_nrt_profile(ids, len(device_ids))
        else:
            rc = lib.axon_start_nrt_profile(None, 0)
        if rc != 0:
            raise RuntimeError(f"axon_start_nrt_profile rc={rc}")
        try:
            yield
        finally:
            n = lib.axon_stop_nrt_profile(str(output_dir).encode())
            print(f"ntff profile: {n} file(s) written to {output_dir}")

    mod.set_axon_ntff_profile_hook(_hook)


_NC_CACHE = None


def _get_module():
    global _NC_CACHE
    if _NC_CACHE is None:
        _NC_CACHE = _build_module()
    return _NC_CACHE


def _run(x, weights, coeff, trace=False):
    nc = _get_module()
    r, consts = _prep_operands(weights, coeff)
    xs = _prep_x(np.asarray(x, dtype=np.float32))
    in_maps = [
        {"xt": xs[core], "r": r, "consts": consts} for core in range(NCORES)
    ]
    res = run_bass_kernel_spmd(nc, in_maps, core_ids=list(range(NCORES)), trace=trace)
    out = np.concatenate(
        [
            res.results[core]["out"]
            .reshape(P, BT, O)
            .transpose(1, 0, 2)
            .reshape(BC, O)
            for core in range(NCORES)
        ],
        axis=0,
    )
    return out, res


def kernel(x, weights, coeff):
    out, _ = _run(x, weights, coeff, trace=False)
    return out


def kernel_traced(x, weights, coeff):
    _install_ntff_hook()
    out, res = _run(x, weights, coeff, trace=True)
    return out, res


# revision 29
# speedup vs baseline: 1.1222x; 1.1222x over previous
"""Trainium2 Bass kernel for a custom Jacobi-basis layer.

Math:
    t = tanh(x)                                  x: [B, I] f32
    J[b,i,k] = P_k^(1,1)(t[b,i])                 Jacobi polys, k = 0..8
    out[b,o] = sum_{i,k} J[b,i,k] * coeff[o,i,k] * weights[o,i]

Strategy (8 NeuronCores, data-parallel over batch):
  * Fold weights into coeff on host: Cw[o,i,k] = coeff[o,i,k]*weights[o,i].
  * alpha=beta=1 makes the three-term recurrence two-term coefficient-free
    after rescaling: G_1 = t, G_k = t*G_{k-1} - B'_k*G_{k-2} with G_k = c_k*J_k.
    The 1/c_k scale is folded into the (host-prepared) matmul operand.
  * J_0 == 1, so the k=0 term is a per-output bias, applied with a K=1 matmul.
  * Per core: tanh/square on ScalarE, fp32 recurrence on VectorE (fused
    scalar_tensor_tensor ops), then 128 float32r matmuls [128x128]@[128x512]
    accumulating in PSUM over the 4096-long (i,k) contraction.
    float32r runs at bf16 speed for N>=256 but has ~1.5e-4 matmul error
    (vs 2.3e-3 for bf16).
"""

import numpy as np

import concourse.mybir as mybir
import concourse.tile as tile
from concourse import bacc
from concourse.bass_utils import run_bass_kernel_spmd

ORDER = 8
B, I, O = 4096, 512, 512
NCORES = 8
BC = B // NCORES          # batch rows per core = 512
P = 128                   # partitions
NIC = I // P              # i-chunks = 4
BT = BC // P              # b-tiles per core = 4
FREE = NIC * BC           # free dim of basis planes = 2048


def _consts():
    """Recurrence constants (alpha=beta=1, so the k2 term is 0)."""
    a = b = 1.0
    A, Bk = {}, {}
    for i in range(2, ORDER + 1):
        A[i] = (2 * i + a + b) * (2 * i + a + b - 1) / (2 * i * (i + a + b))
        Bk[i] = (i + a - 1) * (i + b - 1) * (2 * i + a + b) / (
            i * (i + a + b) * (2 * i + a + b - 2)
        )
    c = {0: 1.0, 1: 0.5}
    for i in range(2, ORDER + 1):
        c[i] = c[i - 1] / A[i]
    Bp = {i: Bk[i] * c[i] / c[i - 2] for i in range(2, ORDER + 1)}
    return c, Bp


def _build_module():
    nc = bacc.Bacc("TRN2", num_devices=NCORES)
    f32 = mybir.dt.float32
    f16 = mybir.dt.float16

    xt_d = nc.dram_tensor("xt", [P, FREE], f32, kind="ExternalInput")
    # r layout: [p, (k-1)*FREE + ic*O + o] = Cw[o, ic*128+p, k] / c_k
    r_d = nc.dram_tensor("r", [P, ORDER * FREE], f16, kind="ExternalInput")
    # consts = [ones(128) | bias(512)]
    consts_d = nc.dram_tensor("consts", [1, P + O], f16, kind="ExternalInput")
    # out layout: [p, bt*O + o] = output[core*BC + bt*128 + p, o]
    out_d = nc.dram_tensor("out", [P, BT * O], f32, kind="ExternalOutput")

    _, Bp = _consts()
    mult = mybir.AluOpType.mult
    add = mybir.AluOpType.add

    from concourse.tile_rust import add_dep_helper

    with tile.TileContext(nc) as tc:
        with (
            tc.tile_pool(name="io", bufs=1) as io,
            tc.tile_pool(name="g", bufs=1) as gp,
            tc.tile_pool(name="u", bufs=2) as up,
            tc.tile_pool(name="psum", bufs=1, space="PSUM") as pp,
        ):
            # DMAs, laddered so early-needed transfers get bandwidth first:
            # consts | xt -> r0 -> r2 -> ...; xt -> r1 -> r3 -> ... (depth 2)
            const_t = io.tile([1, P + O], f16, tag="consts")
            nc.sync.dma_start(const_t[:], consts_d[:])
            ones_t = const_t[:, 0:P]
            bias_t = const_t[:, P : P + O]
            x_t = io.tile([P, FREE], f32, tag="x")
            d_x = nc.sync.dma_start(x_t[:], xt_d[:])
            r_t = []
            d_prev = [d_x, d_x]
            for k in range(ORDER):
                rt = io.tile([P, FREE], f16, tag=f"r{k}", name=f"r{k}")
                d = nc.sync.dma_start(rt[:], r_d[:, k * FREE : (k + 1) * FREE])
                add_dep_helper(d.ins, d_prev[k % 2].ins, reason="dma ladder")
                d_prev[k % 2] = d
                r_t.append(rt)

            # Basis planes G_1..G_8: recurrence in fp32 (DVE), then one
            # rounding cast per plane to fp16 on the (otherwise idle) ScalarE.
            # Casts are emitted per-ic chunk so PE can start each plane early.
            g = [None] * (ORDER + 1)   # fp32 recurrence planes
            gr = [None] * (ORDER + 1)  # fp16 matmul operand planes

            def cast_plane(k, src):
                grk = gp.tile([P, FREE], f16, tag="gr", name=f"gr{k}", bufs=4)
                for ic in range(NIC):
                    nc.scalar.copy(
                        grk[:, ic * BC : (ic + 1) * BC],
                        src[:, ic * BC : (ic + 1) * BC],
                    )
                gr[k] = grk

            t = gp.tile([P, FREE], f32, tag="t")
            nc.scalar.activation(t[:], x_t[:], mybir.ActivationFunctionType.Tanh)
            g[1] = t
            cast_plane(1, t)
            sq = up.tile([P, FREE], f32, tag="sq")
            nc.scalar.square(sq[:], t[:])
            # g2 = s - B'_2 off the DVE chain (ScalarE Copy applies scale+bias)
            g2 = gp.tile([P, FREE], f32, tag="g", name="g2", bufs=3)
            nc.scalar.activation(
                g2[:], sq[:], mybir.ActivationFunctionType.Copy, bias=-Bp[2]
            )
            g[2] = g2
            cast_plane(2, g2)
            # u3 = t*G_2 = (s - B'_2)*t directly from s and t (skips g2 dep)
            u3 = up.tile([P, FREE], f32, tag="u", name="u3")
            nc.vector.scalar_tensor_tensor(u3[:], sq[:], -Bp[2], t[:], add, mult)
            g3 = gp.tile([P, FREE], f32, tag="g", name="g3", bufs=3)
            nc.vector.scalar_tensor_tensor(g3[:], t[:], -Bp[3], u3[:], mult, add)
            g[3] = g3
            cast_plane(3, g3)
            for k in range(4, ORDER + 1):
                u = up.tile([P, FREE], f32, tag="u", name=f"u{k}")
                nc.vector.tensor_tensor(u[:], t[:], g[k - 1][:], mult)
                gk = gp.tile([P, FREE], f32, tag="g", name=f"g{k}", bufs=3)
                nc.vector.scalar_tensor_tensor(
                    gk[:], g[k - 2][:], -Bp[k], u[:], mult, add
                )
                g[k] = gk
                cast_plane(k, gk)

            # Matmuls: psum[bt] = ones^T @ bias + sum_{k,ic} G_k_slice^T @ R_k_slice
            psums = [
                pp.tile([P, O], f32, tag=f"ps{bt}", name=f"ps{bt}")
                for bt in range(BT)
            ]
            # HAM warmup: keep PE busy from the moment consts land so the
            # clock gate is released before the real matmul stream begins.
            ps_warm = pp.tile([P, O], f32, tag="warm", name="ps_warm")
            for w in range(12):
                nc.tensor.matmul(ps_warm[:], ones_t, bias_t, start=True, stop=True)
            for bt in range(BT):
                nc.tensor.matmul(
                    psums[bt][:], ones_t, bias_t, start=True, stop=False
                )
            out_t = io.tile([P, BT * O], f32, tag="out")
            for k in range(1, ORDER + 1):
                if k < ORDER:
                    for ic in range(NIC):
                        for bt in range(BT):
                            col = ic * BC + bt * P
                            nc.tensor.matmul(
                                psums[bt][:],
                                gr[k][:, col : col + P],
                                r_t[k - 1][:, ic * O : (ic + 1) * O],
                                start=False,
                                stop=False,
                            )
                else:
                    # last contraction block: finish b-tiles one at a time so
                    # the psum evictions/stores overlap the remaining matmuls
                    for bt in range(BT):
                        for ic in range(NIC):
                            col = ic * BC + bt * P
                            nc.tensor.matmul(
                                psums[bt][:],
                                gr[k][:, col : col + P],
                                r_t[k - 1][:, ic * O : (ic + 1) * O],
                                start=False,
                                stop=ic == NIC - 1,
                            )
                        dst = out_t[:, bt * O : (bt + 1) * O]
                        if bt % 2 == 0:
                            nc.scalar.copy(dst, psums[bt][:])
                        else:
                            nc.vector.tensor_copy(dst, psums[bt][:])
                        nc.gpsimd.dma_start(
                            out_d[:, bt * O : (bt + 1) * O],
                            out_t[:, bt * O : (bt + 1) * O],
                        )
    nc.compile()
    return nc


def _prep_operands(weights, coeff):
    """Host-side, input-independent preprocessing of the layer constants."""
    c, _ = _consts()
    Cw = coeff.astype(np.float64) * weights.astype(np.float64)[:, :, None]
    bias = Cw[:, :, 0].sum(axis=1)                      # [O]
    r = np.empty((ORDER, P, FREE), dtype=np.float32)
    for k in range(1, ORDER + 1):
        tmp = (Cw[:, :, k] / c[k]).T.astype(np.float32)  # [I, O]
        r[k - 1] = tmp.reshape(NIC, P, O).transpose(1, 0, 2).reshape(P, FREE)
    r = np.ascontiguousarray(
        r.transpose(1, 0, 2).reshape(P, ORDER * FREE)
    ).astype(np.float16)
    consts = np.concatenate(
        [np.ones(P, dtype=np.float32), bias.astype(np.float32)]
    ).reshape(1, P + O).astype(np.float16)
    return r, consts


def _prep_x(x):
    """Per-core [128, FREE] views of x^T: xt[p, ic*BC + b] = x[core*BC+b, ic*128+p]."""
    shards = []
    for core in range(NCORES):
        xc = np.ascontiguousarray(x[core * BC : (core + 1) * BC, :].T)  # [I, BC]
        shards.append(
            np.ascontiguousarray(
                xc.reshape(NIC, P, BC).transpose(1, 0, 2).reshape(P, FREE)
            )
        )
    return shards


def _install_ntff_hook():
    """Register the NTFF profile hook that the image's boot skips (no
    antenv.axon_hooks module). Same ctypes ABI as trn_boot's
    _ntff_profile_via_ctypes. Only used for traced (profiling) runs."""
    import sys
    import types
    import ctypes
    import contextlib

    if "antenv.axon_hooks" in sys.modules:
        return
    mod = types.ModuleType("antenv.axon_hooks")
    state = {"hook": None}
    mod.set_axon_ntff_profile_hook = lambda h: state.__setitem__("hook", h)
    mod.get_axon_ntff_profile_hook = lambda: state["hook"]
    sys.modules["antenv.axon_hooks"] = mod
    import antenv

    antenv.axon_hooks = mod

    so_path = "/opt/axon/libaxon_pjrt.so"
    lib = ctypes.CDLL(so_path)
    if not hasattr(lib, "axon_start_nrt_profile"):
        return
    lib.axon_start_nrt_profile.argtypes = [
        ctypes.POINTER(ctypes.c_int64),
        ctypes.c_size_t,
    ]
    lib.axon_start_nrt_profile.restype = ctypes.c_int64
    lib.axon_stop_nrt_profile.argtypes = [ctypes.c_char_p]
    lib.axon_stop_nrt_profile.restype = ctypes.c_int64

    @contextlib.contextmanager
    def _hook(output_dir, device_ids):
        import jax

        jax.devices()
        if device_ids:
            ids = (ctypes.c_int64 * len(device_ids))(*device_ids)
            rc = lib.axon_start_nrt_profile(ids, len(device_ids))
        else:
            rc = lib.axon_start_nrt_profile(None, 0)
        if rc != 0:
            raise RuntimeError(f"axon_start_nrt_profile rc={rc}")
        try:
            yield
        finally:
            n = lib.axon_stop_nrt_profile(str(output_dir).encode())
            print(f"ntff profile: {n} file(s) written to {output_dir}")

    mod.set_axon_ntff_profile_hook(_hook)


_NC_CACHE = None


def _get_module():
    global _NC_CACHE
    if _NC_CACHE is None:
        _NC_CACHE = _build_module()
    return _NC_CACHE


def _run(x, weights, coeff, trace=False):
    nc = _get_module()
    r, consts = _prep_operands(weights, coeff)
    xs = _prep_x(np.asarray(x, dtype=np.float32))
    in_maps = [
        {"xt": xs[core], "r": r, "consts": consts} for core in range(NCORES)
    ]
    res = run_bass_kernel_spmd(nc, in_maps, core_ids=list(range(NCORES)), trace=trace)
    out = np.concatenate(
        [
            res.results[core]["out"]
            .reshape(P, BT, O)
            .transpose(1, 0, 2)
            .reshape(BC, O)
            for core in range(NCORES)
        ],
        axis=0,
    )
    return out, res


def kernel(x, weights, coeff):
    out, _ = _run(x, weights, coeff, trace=False)
    return out


def kernel_traced(x, weights, coeff):
    _install_ntff_hook()
    out, res = _run(x, weights, coeff, trace=True)
    return out, res


# revision 31
# speedup vs baseline: 1.2030x; 1.0721x over previous
"""Trainium2 Bass kernel for a custom Jacobi-basis layer.

Math:
    t = tanh(x)                                  x: [B, I] f32
    J[b,i,k] = P_k^(1,1)(t[b,i])                 Jacobi polys, k = 0..8
    out[b,o] = sum_{i,k} J[b,i,k] * coeff[o,i,k] * weights[o,i]

Strategy (8 NeuronCores, data-parallel over batch):
  * Fold weights into coeff on host: Cw[o,i,k] = coeff[o,i,k]*weights[o,i].
  * alpha=beta=1 makes the three-term recurrence two-term coefficient-free
    after rescaling: G_1 = t, G_k = t*G_{k-1} - B'_k*G_{k-2} with G_k = c_k*J_k.
    The 1/c_k scale is folded into the (host-prepared) matmul operand.
  * J_0 == 1, so the k=0 term is a per-output bias, applied with a K=1 matmul.
  * Per core: tanh/square on ScalarE, fp32 recurrence on VectorE (fused
    scalar_tensor_tensor ops), then 128 float32r matmuls [128x128]@[128x512]
    accumulating in PSUM over the 4096-long (i,k) contraction.
    float32r runs at bf16 speed for N>=256 but has ~1.5e-4 matmul error
    (vs 2.3e-3 for bf16).
"""

import numpy as np

import concourse.mybir as mybir
import concourse.tile as tile
from concourse import bacc
from concourse.bass_utils import run_bass_kernel_spmd

ORDER = 8
B, I, O = 4096, 512, 512
NCORES = 8
BC = B // NCORES          # batch rows per core = 512
P = 128                   # partitions
NIC = I // P              # i-chunks = 4
BT = BC // P              # b-tiles per core = 4
FREE = NIC * BC           # free dim of basis planes = 2048


def _consts():
    """Recurrence constants (alpha=beta=1, so the k2 term is 0)."""
    a = b = 1.0
    A, Bk = {}, {}
    for i in range(2, ORDER + 1):
        A[i] = (2 * i + a + b) * (2 * i + a + b - 1) / (2 * i * (i + a + b))
        Bk[i] = (i + a - 1) * (i + b - 1) * (2 * i + a + b) / (
            i * (i + a + b) * (2 * i + a + b - 2)
        )
    c = {0: 1.0, 1: 0.5}
    for i in range(2, ORDER + 1):
        c[i] = c[i - 1] / A[i]
    Bp = {i: Bk[i] * c[i] / c[i - 2] for i in range(2, ORDER + 1)}
    return c, Bp


def _build_module():
    nc = bacc.Bacc("TRN2", num_devices=NCORES)
    f32 = mybir.dt.float32
    f16 = mybir.dt.float16

    xt_d = nc.dram_tensor("xt", [P, FREE], f32, kind="ExternalInput")
    # r layout: [p, (k-1)*FREE + ic*O + o] = Cw[o, ic*128+p, k] / c_k
    r_d = nc.dram_tensor("r", [P, ORDER * FREE], f16, kind="ExternalInput")
    # consts row 0 = [ones(128) | bias(512)]; rows 1..127 warmup junk
    consts_d = nc.dram_tensor("consts", [P, P + O], f16, kind="ExternalInput")
    # out layout: [p, bt*O + o] = output[core*BC + bt*128 + p, o]
    out_d = nc.dram_tensor("out", [P, BT * O], f32, kind="ExternalOutput")

    _, Bp = _consts()
    mult = mybir.AluOpType.mult
    add = mybir.AluOpType.add

    from concourse.tile_rust import add_dep_helper

    with tile.TileContext(nc) as tc:
        with (
            tc.tile_pool(name="io", bufs=1) as io,
            tc.tile_pool(name="g", bufs=1) as gp,
            tc.tile_pool(name="u", bufs=2) as up,
            tc.tile_pool(name="psum", bufs=1, space="PSUM") as pp,
        ):
            H = FREE // 2
            halves = (slice(0, H), slice(H, FREE))

            # consts first (tiny; also feeds the PE warmup), then xt in two
            # chained halves, then the r planes on a depth-2 ladder.
            const_t = io.tile([P, P + O], f16, tag="consts")
            nc.sync.dma_start(const_t[:], consts_d[:])
            ones_t = const_t[0:1, 0:P]
            bias_t = const_t[0:1, P : P + O]
            x_t = io.tile([P, FREE], f32, tag="x")
            d_xl = nc.sync.dma_start(x_t[:, halves[0]], xt_d[:, halves[0]])
            d_xr = nc.sync.dma_start(x_t[:, halves[1]], xt_d[:, halves[1]])
            add_dep_helper(d_xr.ins, d_xl.ins, reason="dma ladder")
            r_t = []
            d_prev = [d_xr, d_xr]
            for k in range(ORDER):
                rt = io.tile([P, FREE], f16, tag=f"r{k}", name=f"r{k}")
                d = nc.sync.dma_start(rt[:], r_d[:, k * FREE : (k + 1) * FREE])
                add_dep_helper(d.ins, d_prev[k % 2].ins, reason="dma ladder")
                d_prev[k % 2] = d
                r_t.append(rt)

            # Basis planes G_1..G_8: recurrence in fp32 on VectorE, split into
            # left/right column halves (independent chains) so the left half
            # starts as soon as the left half of x lands. Each fp32 plane is
            # rounded to fp16 per-ic-chunk on ScalarE for the matmuls; G_8 is
            # written in fp16 directly (nothing downstream needs it in fp32).
            g = [None] * (ORDER + 1)
            gr = [None] * (ORDER + 1)

            t = gp.tile([P, FREE], f32, tag="t")
            sq = up.tile([P, FREE], f32, tag="sq")
            for h in (0, 1):
                nc.scalar.activation(
                    t[:, halves[h]],
                    x_t[:, halves[h]],
                    mybir.ActivationFunctionType.Tanh,
                )
                nc.scalar.square(sq[:, halves[h]], t[:, halves[h]])
            g[1] = t

            def cast_chunks(k, src, h):
                for ic in (0, 1) if h == 0 else (2, 3):
                    nc.scalar.copy(
                        gr[k][:, ic * BC : (ic + 1) * BC],
                        src[:, ic * BC : (ic + 1) * BC],
                    )

            gr[1] = gp.tile([P, FREE], f16, tag="gr", name="gr1", bufs=4)
            for h in (0, 1):
                cast_chunks(1, t, h)
            # g2 = s - B2 on ScalarE (off the DVE chain)
            g2 = gp.tile([P, FREE], f32, tag="g", name="g2", bufs=3)
            gr[2] = gp.tile([P, FREE], f16, tag="gr", name="gr2", bufs=4)
            for h in (0, 1):
                nc.scalar.activation(
                    g2[:, halves[h]],
                    sq[:, halves[h]],
                    mybir.ActivationFunctionType.Copy,
                    bias=-Bp[2],
                )
                cast_chunks(2, g2, h)
            g[2] = g2

            # DVE chain, interleaved L/R. u3 = (s - B2)*t skips the g2 dep.
            u3 = up.tile([P, FREE], f32, tag="u", name="u3")
            g3 = gp.tile([P, FREE], f32, tag="g", name="g3", bufs=3)
            gr[3] = gp.tile([P, FREE], f16, tag="gr", name="gr3", bufs=4)
            for h in (0, 1):
                sl = halves[h]
                nc.vector.scalar_tensor_tensor(
                    u3[:, sl], sq[:, sl], -Bp[2], t[:, sl], add, mult
                )
            for h in (0, 1):
                sl = halves[h]
                nc.vector.scalar_tensor_tensor(
                    g3[:, sl], t[:, sl], -Bp[3], u3[:, sl], mult, add
                )
                cast_chunks(3, g3, h)
            g[3] = g3
            for k in range(4, ORDER + 1):
                u = up.tile([P, FREE], f32, tag="u", name=f"u{k}")
                last = k == ORDER
                gk = (
                    gp.tile([P, FREE], f16, tag="gr", name=f"g{k}", bufs=4)
                    if last
                    else gp.tile([P, FREE], f32, tag="g", name=f"g{k}", bufs=3)
                )
                if not last:
                    gr[k] = gp.tile(
                        [P, FREE], f16, tag="gr", name=f"gr{k}", bufs=4
                    )
                for h in (0, 1):
                    sl = halves[h]
                    nc.vector.tensor_tensor(u[:, sl], t[:, sl], g[k - 1][:, sl], mult)
                for h in (0, 1):
                    sl = halves[h]
                    nc.vector.scalar_tensor_tensor(
                        gk[:, sl], g[k - 2][:, sl], -Bp[k], u[:, sl], mult, add
                    )
                    if not last:
                        cast_chunks(k, gk, h)
                g[k] = gk
                if last:
                    gr[k] = gk

            # Matmuls: psum[bt] = ones^T @ bias + sum_{k,ic} G_k_slice^T @ R_k_slice
            psums = [
                pp.tile([P, O], f32, tag=f"ps{bt}", name=f"ps{bt}")
                for bt in range(BT)
            ]
            # HAM warmup with real K=128 matmuls on the consts block so the
            # clock gate is released before the real stream begins.
            ps_warm = pp.tile([P, O], f32, tag="warm", name="ps_warm")
            for w in range(12):
                nc.tensor.matmul(
                    ps_warm[:],
                    const_t[:, 0:P],
                    const_t[:, P : P + O],
                    start=True,
                    stop=True,
                )
            for bt in range(BT):
                nc.tensor.matmul(
                    psums[bt][:], ones_t, bias_t, start=True, stop=False
                )
            out_t = io.tile([P, BT * O], f32, tag="out")
            for k in range(1, ORDER + 1):
                if k < ORDER:
                    for ic in range(NIC):
                        for bt in range(BT):
                            col = ic * BC + bt * P
                            nc.tensor.matmul(
                                psums[bt][:],
                                gr[k][:, col : col + P],
                                r_t[k - 1][:, ic * O : (ic + 1) * O],
                                start=False,
                                stop=False,
                            )
                else:
                    # last block: finish b-tiles one at a time so the psum
                    # evictions/stores overlap the remaining matmuls
                    for bt in range(BT):
                        for ic in range(NIC):
                            col = ic * BC + bt * P
                            nc.tensor.matmul(
                                psums[bt][:],
                                gr[k][:, col : col + P],
                                r_t[k - 1][:, ic * O : (ic + 1) * O],
                                start=False,
                                stop=ic == NIC - 1,
                            )
                        dst = out_t[:, bt * O : (bt + 1) * O]
                        if bt % 2 == 0:
                            nc.scalar.copy(dst, psums[bt][:])
                        else:
                            nc.vector.tensor_copy(dst, psums[bt][:])
                        nc.sync.dma_start(
                            out_d[:, bt * O : (bt + 1) * O],
                            out_t[:, bt * O : (bt + 1) * O],
                        )
    nc.compile()
    return nc


def _prep_operands(weights, coeff):
    """Host-side, input-independent preprocessing of the layer constants."""
    c, _ = _consts()
    Cw = coeff.astype(np.float64) * weights.astype(np.float64)[:, :, None]
    bias = Cw[:, :, 0].sum(axis=1)                      # [O]
    r = np.empty((ORDER, P, FREE), dtype=np.float32)
    for k in range(1, ORDER + 1):
        tmp = (Cw[:, :, k] / c[k]).T.astype(np.float32)  # [I, O]
        r[k - 1] = tmp.reshape(NIC, P, O).transpose(1, 0, 2).reshape(P, FREE)
    r = np.ascontiguousarray(
        r.transpose(1, 0, 2).reshape(P, ORDER * FREE)
    ).astype(np.float16)
    consts = np.ones((P, P + O), dtype=np.float32)
    consts[0, P:] = bias
    consts[1:, :] = 0.5
    return r, consts.astype(np.float16)


def _prep_x(x):
    """Per-core [128, FREE] views of x^T: xt[p, ic*BC + b] = x[core*BC+b, ic*128+p]."""
    shards = []
    for core in range(NCORES):
        xc = np.ascontiguousarray(x[core * BC : (core + 1) * BC, :].T)  # [I, BC]
        shards.append(
            np.ascontiguousarray(
                xc.reshape(NIC, P, BC).transpose(1, 0, 2).reshape(P, FREE)
            )
        )
    return shards


def _install_ntff_hook():
    """Register the NTFF profile hook that the image's boot skips (no
    antenv.axon_hooks module). Same ctypes ABI as trn_boot's
    _ntff_profile_via_ctypes. Only used for traced (profiling) runs."""
    import sys
    import types
    import ctypes
    import contextlib

    if "antenv.axon_hooks" in sys.modules:
        return
    mod = types.ModuleType("antenv.axon_hooks")
    state = {"hook": None}
    mod.set_axon_ntff_profile_hook = lambda h: state.__setitem__("hook", h)
    mod.get_axon_ntff_profile_hook = lambda: state["hook"]
    sys.modules["antenv.axon_hooks"] = mod
    import antenv

    antenv.axon_hooks = mod

    so_path = "/opt/axon/libaxon_pjrt.so"
    lib = ctypes.CDLL(so_path)
    if not hasattr(lib, "axon_start_nrt_profile"):
        return
    lib.axon_start_nrt_profile.argtypes = [
        ctypes.POINTER(ctypes.c_int64),
        ctypes.c_size_t,
    ]
    lib.axon_start_nrt_profile.restype = ctypes.c_int64
    lib.axon_stop_nrt_profile.argtypes = [ctypes.c_char_p]
    lib.axon_stop_nrt_profile.restype = ctypes.c_int64

    @contextlib.contextmanager
    def _hook(output_dir, device_ids):
        import jax

        jax.devices()
        if device_ids:
            ids = (ctypes.c_int64 * len(device_ids))(*device_ids)
            rc = lib.axon_start_nrt_profile(ids, len(device_ids))
        else:
            rc = lib.axon_start_nrt_profile(None, 0)
        if rc != 0:
            raise RuntimeError(f"axon_start_nrt_profile rc={rc}")
        try:
            yield
        finally:
            n = lib.axon_stop_nrt_profile(str(output_dir).encode())
            print(f"ntff profile: {n} file(s) written to {output_dir}")

    mod.set_axon_ntff_profile_hook(_hook)


_NC_CACHE = None


def _get_module():
    global _NC_CACHE
    if _NC_CACHE is None:
        _NC_CACHE = _build_module()
    return _NC_CACHE


def _run(x, weights, coeff, trace=False):
    nc = _get_module()
    r, consts = _prep_operands(weights, coeff)
    xs = _prep_x(np.asarray(x, dtype=np.float32))
    in_maps = [
        {"xt": xs[core], "r": r, "consts": consts} for core in range(NCORES)
    ]
    res = run_bass_kernel_spmd(nc, in_maps, core_ids=list(range(NCORES)), trace=trace)
    out = np.concatenate(
        [
            res.results[core]["out"]
            .reshape(P, BT, O)
            .transpose(1, 0, 2)
            .reshape(BC, O)
            for core in range(NCORES)
        ],
        axis=0,
    )
    return out, res


def kernel(x, weights, coeff):
    out, _ = _run(x, weights, coeff, trace=False)
    return out


def kernel_traced(x, weights, coeff):
    _install_ntff_hook()
    out, res = _run(x, weights, coeff, trace=True)
    return out, res
